# revision 19
# baseline (speedup 1.0000x reference)
"""Trainium2 Bass kernel for the AF-2D-MADE autoregressive sampling block.

Strategy:
- Data-parallel over batch: 16 samples -> 8 NeuronCores x 2 samples, no
  collectives; host shards inputs and concatenates outputs.
- Wavefront scheduling: pixels with equal t = 2i + j are independent (the
  masked-conv receptive field at (i,j) only reaches row i-r up to column j+r,
  and column j-1 within row i), so the 64-pixel raster scan collapses to 22
  sequential wavefront steps updating up to 4 pixels each.
- Both conv networks (mu, lv) are fused into single matmuls with
  block-diagonal weights (64+64 channels on the 128-partition contraction).
- Convs are implicit GEMMs over mask taps: activations live in SBUF as
  [chan, (10, 10, B)] zero/one-padded images so each tap is a strided AP read.
- ELU is computed in the u = elu(h)+1 representation:
      u = max(h + c + 1, min(exp(h + c), 1))
  (exact since exp(x) >= x+1, and |h| ~ 3 so exp never overflows), with
  pad ring = 1.0 and the -sum(W) bias corrections c folded in, so each stage
  is 1 ACT op + 2 DVE ops.
- Matmuls run in fp16 (fp32 PE matmul is ~4x slower: no FWL + half-rate
  streaming). The logstd SUM is cancellation-sensitive (192 correlated
  terms), so it is recomputed at the end by a one-time fp32 lv-net replay
  over the final y. (Valid because no lv output depends on y[7,7]: the
  masks exclude self/raster-later pixels, so the replay on fully-updated y
  equals the reference's final-step logstd exactly.)
"""

import numpy as np
from contextlib import ExitStack

import concourse.bacc as bacc
import concourse.bass as bass
import concourse.mybir as mybir
import concourse.tile as tile
from concourse.bass_utils import run_bass_kernel_spmd

N_CORES = 8
BL = 2  # batch per core
F32 = mybir.dt.float32
F16 = mybir.dt.float16
AF = mybir.ActivationFunctionType
ALU = mybir.AluOpType
TAPS_A = [(0, 0), (0, 1), (0, 2), (1, 0)]
TAPS_B = [(0, 0), (0, 1), (0, 2), (1, 0), (1, 1)]

TRACE = False
LAST_RESULT = None
_NC_CACHE = None

# Params are packed into 4 blobs (by partition-count x dtype) so the head
# costs 4 DMA descriptors instead of 18. Each logical param is a slice.
BLOBS = [
    ("b3f16", [3, 512 + 3 * 128], F16),    # w0 | d1,d2,d3 (in row 0)
    ("b3f32", [3, 128 + 256 + 2], F32),    # x_adj | v0 | nc4lvh | c4lv
    ("b128", [128, 1310], F16),            # w1 | w2 | w3m | w3l
    ("b64", [64, 655 + 3 * 64], F32),      # v1 | v2 | v3 | d1l,d2l,d3l (row 0)
]
# name -> (blob, row_slice, col_start, col_end)
PARAM_SLICES = {
    "w0": ("b3f16", (0, 3), 0, 512),
    "d1": ("b3f16", (0, 1), 512, 640),
    "d2": ("b3f16", (0, 1), 640, 768),
    "d3": ("b3f16", (0, 1), 768, 896),
    "x_adj": ("b3f32", (0, 3), 0, 128),
    "v0": ("b3f32", (0, 3), 128, 384),
    "nc4lvh": ("b3f32", (0, 3), 384, 385),
    "c4lv": ("b3f32", (0, 3), 385, 386),
    "w1": ("b128", (0, 128), 0, 640),
    "w2": ("b128", (0, 128), 640, 1280),
    "w3m": ("b128", (0, 128), 1280, 1295),
    "w3l": ("b128", (0, 128), 1295, 1310),
    "v1": ("b64", (0, 64), 0, 320),
    "v2": ("b64", (0, 64), 320, 640),
    "v3": ("b64", (0, 64), 640, 655),
    "d1l": ("b64", (0, 1), 655, 719),
    "d2l": ("b64", (0, 1), 719, 783),
    "d3l": ("b64", (0, 1), 783, 847),
}


def _img(ap):
    """[P, 200] -> [P, h, (w b)] padded-image view; layout is (h, w, b)."""
    return ap.rearrange("p (h wb) -> p h wb", h=10, wb=10 * BL)


def _qb(ap):
    """[P, n*BL] -> [P, q, b] view (b innermost)."""
    n = ap.shape[-1] // BL
    return ap.rearrange("p (q b) -> p q b", b=BL, q=n)


def build_nc():
    nc = bacc.Bacc("TRN2", debug=False, num_devices=N_CORES)
    prm = {}
    for name, shape, dt in BLOBS:
        prm[name] = nc.declare_dram_parameter(name, shape, dt, isOutput=False)
    out_y = nc.declare_dram_parameter("out_y", [3, 64 * BL], F32, isOutput=True)
    out_ls = nc.declare_dram_parameter("out_ls", [1, BL], F32, isOutput=True)

    with ExitStack() as ctx:
        tc = ctx.enter_context(tile.TileContext(nc))
        const = ctx.enter_context(tc.tile_pool(name="const", bufs=1))
        state = ctx.enter_context(tc.tile_pool(name="state", bufs=1))
        tmp = ctx.enter_context(tc.tile_pool(name="tmp", bufs=3))
        psum = ctx.enter_context(tc.tile_pool(name="psum", bufs=1, space="PSUM"))

        # --- load param blobs, expose logical params as slices ---
        blob_t = {}
        for name, shape, dt in BLOBS:
            blob_t[name] = const.tile(shape, dt, tag=name, name=f"sb_{name}")
            nc.sync.dma_start(blob_t[name][:], prm[name][:])
        sb = {}
        for name, (bl, rows, c0, c1) in PARAM_SLICES.items():
            sb[name] = blob_t[bl][rows[0] : rows[1], c0:c1]

        # --- ACT exp-table preload (overlaps the param DMAs) ---
        warm = state.tile([1, 1], F32, tag="warm")
        nc.vector.memset(warm[:], 0.0)
        warm2 = state.tile([1, 1], F32, tag="warm2")
        nc.scalar.activation(warm2[:], warm[:], AF.Exp)

        # --- persistent state ---
        y16 = state.tile([3, BL * 100], F16, tag="y16")
        nc.vector.memset(y16[:], 0.0)
        us = []
        for l in range(3):
            u = state.tile([128, BL * 100], F16, tag=f"u{l + 1}", name=f"u{l + 1}")
            nc.vector.memset(u[:], 1.0)
            us.append(u)

        ones128 = state.tile([1, BL * 64], F32, tag="ones128")
        nc.vector.memset(ones128[:], 1.0)

        # fp32 replay state (filled progressively as y rows finalize)
        y32 = state.tile([3, BL * 100], F32, tag="y32")
        nc.vector.memset(y32[:], 0.0)
        ru = []
        for l in range(3):
            u = state.tile([64, BL * 100], F32, tag=f"ru{l + 1}", name=f"ru{l + 1}")
            nc.vector.memset(u[:], 1.0)
            ru.append(u)
        lsbuf = state.tile([3, BL * 64], F32, tag="lsbuf")

        def replay_chunk(r0, r1):
            """fp32 lv-net replay for output rows r0..r1 (y rows <= r1 final)."""
            yv32 = _img(y32[:])[:, r0 + 1 : r1 + 2, :]
            yv16 = _img(y16[:])[:, r0 + 1 : r1 + 2, :]
            nc.vector.tensor_copy(yv32, yv16)
            r1h = conv(y32, sb["v0"], TAPS_A, 64, "h1", r0, r1, d_row=sb["d1l"])
            elu_stage(r1h, ru[0], "r", r0, r1)
            r2h = conv(ru[0], sb["v1"], TAPS_B, 64, "h2", r0, r1, d_row=sb["d2l"])
            elu_stage(r2h, ru[1], "r", r0, r1)
            r3h = conv(ru[1], sb["v2"], TAPS_B, 64, "h3", r0, r1, d_row=sb["d3l"])
            elu_stage(r3h, ru[2], "r", r0, r1)
            olvr = conv(ru[2], sb["v3"], TAPS_B, 3, "olv", r0, r1, pbufs=1)
            nc.vector.tensor_scalar(
                lsbuf[:, r0 * 16 : (r1 + 1) * 16],
                olvr[:],
                sb["c4lv"],
                0.5,
                ALU.add,
                ALU.mult,
            )

        def conv(src, wt, taps, m_out, ptag, r0, r1, pbufs=2, d_row=None):
            """Row-ranged conv: output rows r0..r1 -> PSUM [m_out, (r1-r0+1)*16]."""
            nw = (r1 - r0 + 1) * 16
            h = psum.tile([m_out, nw], F32, tag=ptag, bufs=pbufs, name=ptag)
            if d_row is not None:
                nc.tensor.matmul(h[:], d_row, ones128[:, :nw], start=True, stop=False)
            for k, (ky, kx) in enumerate(taps):
                rhs = _img(src[:])[:, ky + r0 : ky + r1 + 1, BL * kx : BL * (kx + 8)]
                nc.tensor.matmul(
                    h[:],
                    wt[:, k * m_out : (k + 1) * m_out],
                    rhs,
                    start=(d_row is None and k == 0),
                    stop=(k == len(taps) - 1),
                )
            return h

        def elu_stage(h, u_out, pfx, r0, r1):
            """u_out interior rows r0..r1 <- elu(h+c)+1 = max(psum, min(exp(psum-1),1));
            psum already contains h+c+1 via the const-tap matmul."""
            p = h.shape[0]
            ex = tmp.tile([p, (r1 - r0 + 1) * 16], F32, tag=f"{pfx}ex", name=f"{pfx}ex")
            nc.scalar.activation(ex[:], h[:], AF.Exp, bias=neg1[0:p, :], scale=1.0)
            u_int = _img(u_out[:])[:, r0 + 1 : r1 + 2, BL : 9 * BL]
            nc.vector.scalar_tensor_tensor(u_int, ex[:], 1.0, h[:], ALU.min, ALU.max)

        # --- 22 wavefront steps (all-fp16 matmuls) ---
        # Cone restriction: each layer's activations are FINAL when computed
        # at their own wavefront (they depend only on raster-earlier pixels),
        # so per step we compute h/u only at the <=4 wavefront pixels (N<=8)
        # and cache them; taps of later steps read the cached values.
        ones8 = state.tile([1, 2 * 4], F16, tag="ones8")
        nc.vector.memset(ones8[:], 1.0)
        neg1 = state.tile([128, 1], F32, tag="neg1")
        nc.vector.memset(neg1[:], -1.0)

        def wf_tap(buf, t, i_min, n_i, ky, kx):
            """[P, n_i, 2] view of padded buf at tap (ky,kx) of wavefront t."""
            q0 = 8 * i_min + t + 10 * ky + kx
            return _qb(buf[:])[:, q0 : q0 + 8 * (n_i - 1) + 1 : 8, :]

        def wf_conv(src_buf, wt, dt_row, taps, m_out, ptag, t, i_min, n_i, pbufs=2):
            """PSUM [m_out, 2*n_i] = sum_taps W_tap @ src(tap) (+ optional d row)."""
            W = 2 * n_i
            h = psum.tile([m_out, W], F32, tag=ptag, bufs=pbufs, name=ptag)
            if dt_row is not None:
                nc.tensor.matmul(h[:], dt_row, ones8[:, :W], start=True, stop=False)
            for k, (ky, kx) in enumerate(taps):
                rhs = wf_tap(src_buf, t, i_min, n_i, ky, kx)
                nc.tensor.matmul(
                    h[:],
                    wt[:, k * m_out : (k + 1) * m_out],
                    rhs,
                    start=(dt_row is None and k == 0),
                    stop=(k == len(taps) - 1),
                )
            return h

        for t in range(22):
            i_min = max(0, (t - 6) // 2)
            i_max = min(7, t // 2)
            n_i = i_max - i_min + 1
            W = 2 * n_i

            for l, (src_buf, wt, dt_row, taps) in enumerate(
                [
                    (y16, sb["w0"], sb["d1"], TAPS_A),
                    (us[0], sb["w1"], sb["d2"], TAPS_B),
                    (us[1], sb["w2"], sb["d3"], TAPS_B),
                ]
            ):
                # psum h = h_conv + c + 1
                h = wf_conv(src_buf, wt, dt_row, taps, 128, f"h{l + 1}", t, i_min, n_i)
                # u[wavefront] = elu(h_conv + c) + 1 = max(psum, min(exp(psum - 1), 1))
                ex = tmp.tile([128, W], F32, tag="aex", name="aex")
                nc.scalar.activation(ex[:], h[:], AF.Exp, bias=neg1[:], scale=1.0)
                q0 = 8 * i_min + t + 11
                u_int = _qb(us[l][:])[:, q0 : q0 + 8 * (n_i - 1) + 1 : 8, :]
                nc.vector.scalar_tensor_tensor(u_int, ex[:], 1.0, h[:], ALU.min, ALU.max)

            o_mu = wf_conv(us[2], sb["w3m"], None, TAPS_B, 3, "omu", t, i_min, n_i, pbufs=1)
            o_lv = wf_conv(us[2], sb["w3l"], None, TAPS_B, 3, "olv", t, i_min, n_i, pbufs=1)

            # rinv = exp(-0.5*(lv + c4lv)) = 1/exp(logstd)   (ACT, parallel with num)
            rinv = tmp.tile([3, W], F32, tag="rinv", name="rinv")
            nc.scalar.activation(rinv[:], o_lv[:], AF.Exp, bias=sb["nc4lvh"], scale=-0.5)
            # num = x_adj - mu_psum  (x_adj already has -c4mu folded in)
            num = tmp.tile([3, W], F32, tag="num", name="num")
            xc = _qb(sb["x_adj"])[:, t + 6 * i_min : t + 6 * i_min + 6 * (n_i - 1) + 1 : 6, :]
            nc.vector.scalar_tensor_tensor(num[:], o_mu[:], -1.0, xc, ALU.mult, ALU.add)
            # y16[wavefront] = num * rinv
            ywf = _qb(y16[:])[:, 8 * i_min + t + 11 : 8 * i_min + t + 11 + 8 * (n_i - 1) + 1 : 8, :]
            nvw = num[:].rearrange("p (q b) -> p q b", b=BL, q=n_i)
            rvw = rinv[:].rearrange("p (q b) -> p q b", b=BL, q=n_i)
            nc.vector.tensor_tensor(ywf, nvw, rvw, ALU.mult)

            # progressive fp32 replay: rows (r,r+1) final after step t=2r+9
            if t in (9, 13, 17):
                replay_chunk((t - 9) // 2, (t - 9) // 2 + 1)

        # --- epilogue: last replay chunk, outputs ---
        replay_chunk(6, 7)
        y_int = _img(y32[:])[:, 1:9, BL : 9 * BL]
        nc.sync.dma_start(out_y[:], y_int)
        red = state.tile([3, BL], F32, tag="red")
        lsv = lsbuf[:].rearrange("p (q b) -> p b q", b=BL, q=64)
        nc.vector.tensor_reduce(red[:], lsv, mybir.AxisListType.X, ALU.add)
        ones3 = state.tile([3, 1], F32, tag="ones3")
        nc.vector.memset(ones3[:], 1.0)
        lsps = psum.tile([1, BL], F32, tag="omu")
        nc.tensor.matmul(lsps[:], ones3[:], red[:], start=True, stop=True)
        lso = state.tile([1, BL], F32, tag="lso")
        nc.vector.tensor_copy(lso[:], lsps[:])
        nc.sync.dma_start(out_ls[:], lso[:])

    nc.compile()
    return nc


def prep_params(inputs):
    """Host-side preprocessing of weights (shared across cores)."""
    g = {k: np.asarray(v, np.float32) for k, v in inputs.items()}

    def bd(a, b):
        out = np.zeros((a.shape[0] + b.shape[0], a.shape[1] + b.shape[1]), np.float32)
        out[: a.shape[0], : a.shape[1]] = a
        out[a.shape[0] :, a.shape[1] :] = b
        return out

    p = {}
    p["w0"] = np.concatenate(
        [
            np.concatenate(
                [g["mu_w0"][:, :, ky, kx].T, g["lv_w0"][:, :, ky, kx].T], axis=1
            )
            for ky, kx in TAPS_A
        ],
        axis=1,
    ).astype(np.float16)
    for l, name in ((1, "w1"), (2, "w2")):
        p[name] = np.concatenate(
            [
                bd(g[f"mu_w{l}"][:, :, ky, kx].T, g[f"lv_w{l}"][:, :, ky, kx].T)
                for ky, kx in TAPS_B
            ],
            axis=1,
        ).astype(np.float16)
    p["w3m"] = np.concatenate(
        [
            np.vstack([g["mu_w3"][:, :, ky, kx].T, np.zeros((64, 3), np.float32)])
            for ky, kx in TAPS_B
        ],
        axis=1,
    ).astype(np.float16)
    p["w3l"] = np.concatenate(
        [
            np.vstack([np.zeros((64, 3), np.float32), g["lv_w3"][:, :, ky, kx].T])
            for ky, kx in TAPS_B
        ],
        axis=1,
    ).astype(np.float16)
    # fp32 lv-only replay weights
    p["v0"] = np.ascontiguousarray(
        np.concatenate([g["lv_w0"][:, :, ky, kx].T for ky, kx in TAPS_A], axis=1)
    )
    for l, name in ((1, "v1"), (2, "v2"), (3, "v3")):
        p[name] = np.ascontiguousarray(
            np.concatenate([g[f"lv_w{l}"][:, :, ky, kx].T for ky, kx in TAPS_B], axis=1)
        )
    # bias corrections: layer l>=1 input is u-1 with u-pad=1 -> c_l = b_l - sum(W_l)
    c1 = np.concatenate([g["mu_b0"], g["lv_b0"]])
    c2 = np.concatenate(
        [
            g["mu_b1"] - sum(g["mu_w1"][:, :, ky, kx].sum(1) for ky, kx in TAPS_B),
            g["lv_b1"] - sum(g["lv_w1"][:, :, ky, kx].sum(1) for ky, kx in TAPS_B),
        ]
    )
    c3 = np.concatenate(
        [
            g["mu_b2"] - sum(g["mu_w2"][:, :, ky, kx].sum(1) for ky, kx in TAPS_B),
            g["lv_b2"] - sum(g["lv_w2"][:, :, ky, kx].sum(1) for ky, kx in TAPS_B),
        ]
    )
    p["d1"] = (c1 + 1.0).reshape(1, 128).astype(np.float16)
    p["d2"] = (c2 + 1.0).reshape(1, 128).astype(np.float16)
    p["d3"] = (c3 + 1.0).reshape(1, 128).astype(np.float16)
    p["d1l"] = np.ascontiguousarray((c1[64:] + 1.0).reshape(1, 64))
    p["d2l"] = np.ascontiguousarray((c2[64:] + 1.0).reshape(1, 64))
    p["d3l"] = np.ascontiguousarray((c3[64:] + 1.0).reshape(1, 64))
    c4mu = g["mu_b3"] - sum(g["mu_w3"][:, :, ky, kx].sum(1) for ky, kx in TAPS_B)
    c4lv = g["lv_b3"] - sum(g["lv_w3"][:, :, ky, kx].sum(1) for ky, kx in TAPS_B)
    p["c4lv"] = np.ascontiguousarray(c4lv.reshape(3, 1))
    p["nc4lvh"] = np.ascontiguousarray((-0.5 * c4lv).reshape(3, 1))
    p["_c4mu"] = c4mu
    return p


def make_in_maps(inputs):
    p = prep_params(inputs)
    x = np.asarray(inputs["x"], np.float32)

    b3f16 = np.zeros((3, 896), np.float16)
    b3f16[:, 0:512] = p["w0"]
    b3f16[0:1, 512:640] = p["d1"]
    b3f16[0:1, 640:768] = p["d2"]
    b3f16[0:1, 768:896] = p["d3"]
    b128 = np.zeros((128, 1310), np.float16)
    b128[:, 0:640] = p["w1"]
    b128[:, 640:1280] = p["w2"]
    b128[:, 1280:1295] = p["w3m"]
    b128[:, 1295:1310] = p["w3l"]
    b64 = np.zeros((64, 847), np.float32)
    b64[:, 0:320] = p["v1"]
    b64[:, 320:640] = p["v2"]
    b64[:, 640:655] = p["v3"]
    b64[0:1, 655:719] = p["d1l"]
    b64[0:1, 719:783] = p["d2l"]
    b64[0:1, 783:847] = p["d3l"]

    in_maps = []
    for c in range(N_CORES):
        xs = x[c * BL : (c + 1) * BL]
        # layout (c, h, w, b)
        x_adj = xs.transpose(1, 2, 3, 0).reshape(3, 64 * BL) - p["_c4mu"][:, None]
        b3f32 = np.zeros((3, 386), np.float32)
        b3f32[:, 0:128] = x_adj
        b3f32[:, 128:384] = p["v0"]
        b3f32[:, 384:385] = p["nc4lvh"]
        b3f32[:, 385:386] = p["c4lv"]
        in_maps.append(
            {
                "b3f16": b3f16,
                "b3f32": np.ascontiguousarray(b3f32),
                "b128": b128,
                "b64": b64,
            }
        )
    return in_maps


def kernel(**inputs):
    global _NC_CACHE, LAST_RESULT
    if _NC_CACHE is None:
        _NC_CACHE = build_nc()
    in_maps = make_in_maps(inputs)
    res = run_bass_kernel_spmd(
        _NC_CACHE, in_maps, core_ids=list(range(N_CORES)), trace=TRACE
    )
    LAST_RESULT = res
    ys, lss = [], []
    for c in range(N_CORES):
        ys.append(res.results[c]["out_y"].reshape(3, 8, 8, BL).transpose(3, 0, 1, 2))
        lss.append(res.results[c]["out_ls"].reshape(BL))
    return (
        np.ascontiguousarray(np.concatenate(ys), dtype=np.float32),
        np.ascontiguousarray(np.concatenate(lss), dtype=np.float32),
    )


# revision 20
# speedup vs baseline: 1.2941x; 1.2941x over previous
"""Trainium2 Bass kernel for the AF-2D-MADE autoregressive sampling block.

Strategy:
- Data-parallel over batch: 16 samples -> 8 NeuronCores x 2 samples, no
  collectives; host shards inputs and concatenates outputs.
- Wavefront scheduling: pixels with equal t = 2i + j are independent (the
  masked-conv receptive field at (i,j) only reaches row i-r up to column j+r,
  and column j-1 within row i), so the 64-pixel raster scan collapses to 22
  sequential wavefront steps updating up to 4 pixels each.
- Both conv networks (mu, lv) are fused into single matmuls with
  block-diagonal weights (64+64 channels on the 128-partition contraction).
- Convs are implicit GEMMs over mask taps: activations live in SBUF as
  [chan, (10, 10, B)] zero/one-padded images so each tap is a strided AP read.
- ELU is computed in the u = elu(h)+1 representation:
      u = max(h + c + 1, min(exp(h + c), 1))
  (exact since exp(x) >= x+1, and |h| ~ 3 so exp never overflows), with
  pad ring = 1.0 and the -sum(W) bias corrections c folded in, so each stage
  is 1 ACT op + 2 DVE ops.
- Matmuls run in fp16 (fp32 PE matmul is ~4x slower: no FWL + half-rate
  streaming). The logstd SUM is cancellation-sensitive (192 correlated
  terms), so it is recomputed at the end by a one-time fp32 lv-net replay
  over the final y. (Valid because no lv output depends on y[7,7]: the
  masks exclude self/raster-later pixels, so the replay on fully-updated y
  equals the reference's final-step logstd exactly.)
"""

import numpy as np
from contextlib import ExitStack

import concourse.bacc as bacc
import concourse.bass as bass
import concourse.mybir as mybir
import concourse.tile as tile
from concourse.bass_utils import run_bass_kernel_spmd

N_CORES = 8
BL = 2  # batch per core
F32 = mybir.dt.float32
F16 = mybir.dt.float16
AF = mybir.ActivationFunctionType
ALU = mybir.AluOpType
TAPS_A = [(0, 0), (0, 1), (0, 2), (1, 0)]
TAPS_B = [(0, 0), (0, 1), (0, 2), (1, 0), (1, 1)]

TRACE = False
LAST_RESULT = None
_NC_CACHE = None

# Params are packed into 4 blobs (by partition-count x dtype) so the head
# costs 4 DMA descriptors instead of 18. Each logical param is a slice.
BLOBS = [
    ("b3f16", [3, 512 + 3 * 128], F16),    # w0 | d1,d2,d3 (in row 0)
    ("b3f32", [3, 128 + 256 + 2], F32),    # x_adj | v0 | nc4lvh | c4lv
    ("b128", [128, 1310], F16),            # w1 | w2 | w3m | w3l
    ("b64", [64, 655 + 3 * 64], F32),      # v1 | v2 | v3 | d1l,d2l,d3l (row 0)
]
# name -> (blob, row_slice, col_start, col_end)
PARAM_SLICES = {
    "w0": ("b3f16", (0, 3), 0, 512),
    "d1": ("b3f16", (0, 1), 512, 640),
    "d2": ("b3f16", (0, 1), 640, 768),
    "d3": ("b3f16", (0, 1), 768, 896),
    "x_adj": ("b3f32", (0, 3), 0, 128),
    "v0": ("b3f32", (0, 3), 128, 384),
    "nc4lvh": ("b3f32", (0, 3), 384, 385),
    "c4lv": ("b3f32", (0, 3), 385, 386),
    "w1": ("b128", (0, 128), 0, 640),
    "w2": ("b128", (0, 128), 640, 1280),
    "w3m": ("b128", (0, 128), 1280, 1295),
    "w3l": ("b128", (0, 128), 1295, 1310),
    "v1": ("b64", (0, 64), 0, 320),
    "v2": ("b64", (0, 64), 320, 640),
    "v3": ("b64", (0, 64), 640, 655),
    "d1l": ("b64", (0, 1), 655, 719),
    "d2l": ("b64", (0, 1), 719, 783),
    "d3l": ("b64", (0, 1), 783, 847),
}


def _img(ap):
    """[P, 200] -> [P, h, (w b)] padded-image view; layout is (h, w, b)."""
    return ap.rearrange("p (h wb) -> p h wb", h=10, wb=10 * BL)


def _qb(ap):
    """[P, n*BL] -> [P, q, b] view (b innermost)."""
    n = ap.shape[-1] // BL
    return ap.rearrange("p (q b) -> p q b", b=BL, q=n)


def build_nc():
    nc = bacc.Bacc("TRN2", debug=False, num_devices=N_CORES)
    prm = {}
    for name, shape, dt in BLOBS:
        prm[name] = nc.declare_dram_parameter(name, shape, dt, isOutput=False)
    out_y = nc.declare_dram_parameter("out_y", [3, 64 * BL], F32, isOutput=True)
    out_ls = nc.declare_dram_parameter("out_ls", [1, BL], F32, isOutput=True)

    with ExitStack() as ctx:
        tc = ctx.enter_context(tile.TileContext(nc))
        const = ctx.enter_context(tc.tile_pool(name="const", bufs=1))
        state = ctx.enter_context(tc.tile_pool(name="state", bufs=1))
        tmp = ctx.enter_context(tc.tile_pool(name="tmp", bufs=3))
        psum = ctx.enter_context(tc.tile_pool(name="psum", bufs=1, space="PSUM"))

        # --- load param blobs, expose logical params as slices ---
        blob_t = {}
        for name, shape, dt in BLOBS:
            blob_t[name] = const.tile(shape, dt, tag=name, name=f"sb_{name}")
            nc.sync.dma_start(blob_t[name][:], prm[name][:])
        sb = {}
        for name, (bl, rows, c0, c1) in PARAM_SLICES.items():
            sb[name] = blob_t[bl][rows[0] : rows[1], c0:c1]

        # --- ACT exp-table preload (overlaps the param DMAs) ---
        warm = state.tile([1, 1], F32, tag="warm")
        nc.vector.memset(warm[:], 0.0)
        warm2 = state.tile([1, 1], F32, tag="warm2")
        nc.scalar.activation(warm2[:], warm[:], AF.Exp)

        # --- persistent state ---
        y16 = state.tile([3, BL * 100], F16, tag="y16")
        nc.vector.memset(y16[:], 0.0)
        us = []
        for l in range(3):
            u = state.tile([128, BL * 100], F16, tag=f"u{l + 1}", name=f"u{l + 1}")
            nc.vector.memset(u[:], 1.0)
            us.append(u)

        ones128 = state.tile([1, BL * 64], F32, tag="ones128")
        nc.vector.memset(ones128[:], 1.0)

        # fp32 replay state (filled progressively as y rows finalize)
        y32 = state.tile([3, BL * 100], F32, tag="y32")
        nc.vector.memset(y32[:], 0.0)
        ru = []
        for l in range(3):
            u = state.tile([64, BL * 100], F32, tag=f"ru{l + 1}", name=f"ru{l + 1}")
            nc.vector.memset(u[:], 1.0)
            ru.append(u)
        lsbuf = state.tile([3, BL * 64], F32, tag="lsbuf")

        def replay_chunk(r0, r1):
            """fp32 lv-net replay for output rows r0..r1 (y rows <= r1 final)."""
            yv32 = _img(y32[:])[:, r0 + 1 : r1 + 2, :]
            yv16 = _img(y16[:])[:, r0 + 1 : r1 + 2, :]
            nc.vector.tensor_copy(yv32, yv16)
            r1h = conv(y32, sb["v0"], TAPS_A, 64, "h1", r0, r1, d_row=sb["d1l"])
            elu_stage(r1h, ru[0], "r", r0, r1)
            r2h = conv(ru[0], sb["v1"], TAPS_B, 64, "h2", r0, r1, d_row=sb["d2l"])
            elu_stage(r2h, ru[1], "r", r0, r1)
            r3h = conv(ru[1], sb["v2"], TAPS_B, 64, "h3", r0, r1, d_row=sb["d3l"])
            elu_stage(r3h, ru[2], "r", r0, r1)
            olvr = conv(ru[2], sb["v3"], TAPS_B, 3, "olv", r0, r1, pbufs=1)
            nc.vector.tensor_scalar(
                lsbuf[:, r0 * 16 : (r1 + 1) * 16],
                olvr[:],
                sb["c4lv"],
                0.5,
                ALU.add,
                ALU.mult,
            )

        def conv(src, wt, taps, m_out, ptag, r0, r1, pbufs=2, d_row=None):
            """Row-ranged conv: output rows r0..r1 -> PSUM [m_out, (r1-r0+1)*16]."""
            nw = (r1 - r0 + 1) * 16
            h = psum.tile([m_out, nw], F32, tag=ptag, bufs=pbufs, name=ptag)
            if d_row is not None:
                nc.tensor.matmul(h[:], d_row, ones128[:, :nw], start=True, stop=False)
            for k, (ky, kx) in enumerate(taps):
                rhs = _img(src[:])[:, ky + r0 : ky + r1 + 1, BL * kx : BL * (kx + 8)]
                nc.tensor.matmul(
                    h[:],
                    wt[:, k * m_out : (k + 1) * m_out],
                    rhs,
                    start=(d_row is None and k == 0),
                    stop=(k == len(taps) - 1),
                )
            return h

        def elu_stage(h, u_out, pfx, r0, r1):
            """u_out interior rows r0..r1 <- elu(h+c)+1 = max(psum, min(exp(psum-1),1));
            psum already contains h+c+1 via the const-tap matmul."""
            p = h.shape[0]
            ex = tmp.tile([p, (r1 - r0 + 1) * 16], F32, tag=f"{pfx}ex", name=f"{pfx}ex")
            nc.scalar.activation(ex[:], h[:], AF.Exp, bias=neg1[0:p, :], scale=1.0)
            u_int = _img(u_out[:])[:, r0 + 1 : r1 + 2, BL : 9 * BL]
            nc.vector.scalar_tensor_tensor(u_int, ex[:], 1.0, h[:], ALU.min, ALU.max)

        # --- 22 wavefront steps (all-fp16 matmuls) ---
        # Cone restriction: each layer's activations are FINAL when computed
        # at their own wavefront (they depend only on raster-earlier pixels),
        # so per step we compute h/u only at the <=4 wavefront pixels (N<=8)
        # and cache them; taps of later steps read the cached values.
        ones8 = state.tile([1, 2 * 4], F16, tag="ones8")
        nc.vector.memset(ones8[:], 1.0)
        neg1 = state.tile([128, 1], F32, tag="neg1")
        nc.vector.memset(neg1[:], -1.0)

        def wf_tap(buf, t, i_min, n_i, ky, kx):
            """[P, n_i, 2] view of padded buf at tap (ky,kx) of wavefront t."""
            q0 = 8 * i_min + t + 10 * ky + kx
            return _qb(buf[:])[:, q0 : q0 + 8 * (n_i - 1) + 1 : 8, :]

        def wf_conv(src_buf, wt, dt_row, taps, m_out, ptag, t, i_min, n_i, pbufs=2):
            """PSUM [m_out, 2*n_i] = sum_taps W_tap @ src(tap) (+ optional d row)."""
            W = 2 * n_i
            h = psum.tile([m_out, W], F32, tag=ptag, bufs=pbufs, name=ptag)
            if dt_row is not None:
                nc.tensor.matmul(h[:], dt_row, ones8[:, :W], start=True, stop=False)
            for k, (ky, kx) in enumerate(taps):
                rhs = wf_tap(src_buf, t, i_min, n_i, ky, kx)
                nc.tensor.matmul(
                    h[:],
                    wt[:, k * m_out : (k + 1) * m_out],
                    rhs,
                    start=(dt_row is None and k == 0),
                    stop=(k == len(taps) - 1),
                )
            return h

        for t in range(22):
            i_min = max(0, (t - 6) // 2)
            i_max = min(7, t // 2)
            n_i = i_max - i_min + 1
            W = 2 * n_i

            for l, (src_buf, wt, dt_row, taps) in enumerate(
                [
                    (y16, sb["w0"], sb["d1"], TAPS_A),
                    (us[0], sb["w1"], sb["d2"], TAPS_B),
                    (us[1], sb["w2"], sb["d3"], TAPS_B),
                ]
            ):
                # psum h = h_conv + c + 1
                h = wf_conv(src_buf, wt, dt_row, taps, 128, f"h{l + 1}", t, i_min, n_i)
                # u[wavefront] = elu(h_conv + c) + 1 = max(psum, min(exp(psum - 1), 1))
                ex = tmp.tile([128, W], F32, tag="aex", name="aex")
                nc.scalar.activation(ex[:], h[:], AF.Exp, bias=neg1[:], scale=1.0)
                q0 = 8 * i_min + t + 11
                u_int = _qb(us[l][:])[:, q0 : q0 + 8 * (n_i - 1) + 1 : 8, :]
                nc.vector.scalar_tensor_tensor(u_int, ex[:], 1.0, h[:], ALU.min, ALU.max)

            o_mu = wf_conv(us[2], sb["w3m"], None, TAPS_B, 3, "omu", t, i_min, n_i, pbufs=1)
            o_lv = wf_conv(us[2], sb["w3l"], None, TAPS_B, 3, "olv", t, i_min, n_i, pbufs=1)

            # rinv = exp(-0.5*(lv + c4lv)) = 1/exp(logstd)   (ACT, parallel with num)
            rinv = tmp.tile([3, W], F32, tag="rinv", name="rinv")
            nc.scalar.activation(rinv[:], o_lv[:], AF.Exp, bias=sb["nc4lvh"], scale=-0.5)
            # num = x_adj - mu_psum  (x_adj already has -c4mu folded in)
            num = tmp.tile([3, W], F32, tag="num", name="num")
            xc = _qb(sb["x_adj"])[:, t + 6 * i_min : t + 6 * i_min + 6 * (n_i - 1) + 1 : 6, :]
            nc.vector.scalar_tensor_tensor(num[:], o_mu[:], -1.0, xc, ALU.mult, ALU.add)
            # y16[wavefront] = num * rinv
            ywf = _qb(y16[:])[:, 8 * i_min + t + 11 : 8 * i_min + t + 11 + 8 * (n_i - 1) + 1 : 8, :]
            nvw = num[:].rearrange("p (q b) -> p q b", b=BL, q=n_i)
            rvw = rinv[:].rearrange("p (q b) -> p q b", b=BL, q=n_i)
            nc.vector.tensor_tensor(ywf, nvw, rvw, ALU.mult)

        # --- epilogue: fp32 lv replay + outputs ---
        replay_chunk(0, 7)
        y_int = _img(y32[:])[:, 1:9, BL : 9 * BL]
        nc.sync.dma_start(out_y[:], y_int)
        red = state.tile([3, BL], F32, tag="red")
        lsv = lsbuf[:].rearrange("p (q b) -> p b q", b=BL, q=64)
        nc.vector.tensor_reduce(red[:], lsv, mybir.AxisListType.X, ALU.add)
        ones3 = state.tile([3, 1], F32, tag="ones3")
        nc.vector.memset(ones3[:], 1.0)
        lsps = psum.tile([1, BL], F32, tag="omu")
        nc.tensor.matmul(lsps[:], ones3[:], red[:], start=True, stop=True)
        lso = state.tile([1, BL], F32, tag="lso")
        nc.vector.tensor_copy(lso[:], lsps[:])
        nc.sync.dma_start(out_ls[:], lso[:])

    nc.compile()
    return nc


def prep_params(inputs):
    """Host-side preprocessing of weights (shared across cores)."""
    g = {k: np.asarray(v, np.float32) for k, v in inputs.items()}

    def bd(a, b):
        out = np.zeros((a.shape[0] + b.shape[0], a.shape[1] + b.shape[1]), np.float32)
        out[: a.shape[0], : a.shape[1]] = a
        out[a.shape[0] :, a.shape[1] :] = b
        return out

    p = {}
    p["w0"] = np.concatenate(
        [
            np.concatenate(
                [g["mu_w0"][:, :, ky, kx].T, g["lv_w0"][:, :, ky, kx].T], axis=1
            )
            for ky, kx in TAPS_A
        ],
        axis=1,
    ).astype(np.float16)
    for l, name in ((1, "w1"), (2, "w2")):
        p[name] = np.concatenate(
            [
                bd(g[f"mu_w{l}"][:, :, ky, kx].T, g[f"lv_w{l}"][:, :, ky, kx].T)
                for ky, kx in TAPS_B
            ],
            axis=1,
        ).astype(np.float16)
    p["w3m"] = np.concatenate(
        [
            np.vstack([g["mu_w3"][:, :, ky, kx].T, np.zeros((64, 3), np.float32)])
            for ky, kx in TAPS_B
        ],
        axis=1,
    ).astype(np.float16)
    p["w3l"] = np.concatenate(
        [
            np.vstack([np.zeros((64, 3), np.float32), g["lv_w3"][:, :, ky, kx].T])
            for ky, kx in TAPS_B
        ],
        axis=1,
    ).astype(np.float16)
    # fp32 lv-only replay weights
    p["v0"] = np.ascontiguousarray(
        np.concatenate([g["lv_w0"][:, :, ky, kx].T for ky, kx in TAPS_A], axis=1)
    )
    for l, name in ((1, "v1"), (2, "v2"), (3, "v3")):
        p[name] = np.ascontiguousarray(
            np.concatenate([g[f"lv_w{l}"][:, :, ky, kx].T for ky, kx in TAPS_B], axis=1)
        )
    # bias corrections: layer l>=1 input is u-1 with u-pad=1 -> c_l = b_l - sum(W_l)
    c1 = np.concatenate([g["mu_b0"], g["lv_b0"]])
    c2 = np.concatenate(
        [
            g["mu_b1"] - sum(g["mu_w1"][:, :, ky, kx].sum(1) for ky, kx in TAPS_B),
            g["lv_b1"] - sum(g["lv_w1"][:, :, ky, kx].sum(1) for ky, kx in TAPS_B),
        ]
    )
    c3 = np.concatenate(
        [
            g["mu_b2"] - sum(g["mu_w2"][:, :, ky, kx].sum(1) for ky, kx in TAPS_B),
            g["lv_b2"] - sum(g["lv_w2"][:, :, ky, kx].sum(1) for ky, kx in TAPS_B),
        ]
    )
    p["d1"] = (c1 + 1.0).reshape(1, 128).astype(np.float16)
    p["d2"] = (c2 + 1.0).reshape(1, 128).astype(np.float16)
    p["d3"] = (c3 + 1.0).reshape(1, 128).astype(np.float16)
    p["d1l"] = np.ascontiguousarray((c1[64:] + 1.0).reshape(1, 64))
    p["d2l"] = np.ascontiguousarray((c2[64:] + 1.0).reshape(1, 64))
    p["d3l"] = np.ascontiguousarray((c3[64:] + 1.0).reshape(1, 64))
    c4mu = g["mu_b3"] - sum(g["mu_w3"][:, :, ky, kx].sum(1) for ky, kx in TAPS_B)
    c4lv = g["lv_b3"] - sum(g["lv_w3"][:, :, ky, kx].sum(1) for ky, kx in TAPS_B)
    p["c4lv"] = np.ascontiguousarray(c4lv.reshape(3, 1))
    p["nc4lvh"] = np.ascontiguousarray((-0.5 * c4lv).reshape(3, 1))
    p["_c4mu"] = c4mu
    return p


def make_in_maps(inputs):
    p = prep_params(inputs)
    x = np.asarray(inputs["x"], np.float32)

    b3f16 = np.zeros((3, 896), np.float16)
    b3f16[:, 0:512] = p["w0"]
    b3f16[0:1, 512:640] = p["d1"]
    b3f16[0:1, 640:768] = p["d2"]
    b3f16[0:1, 768:896] = p["d3"]
    b128 = np.zeros((128, 1310), np.float16)
    b128[:, 0:640] = p["w1"]
    b128[:, 640:1280] = p["w2"]
    b128[:, 1280:1295] = p["w3m"]
    b128[:, 1295:1310] = p["w3l"]
    b64 = np.zeros((64, 847), np.float32)
    b64[:, 0:320] = p["v1"]
    b64[:, 320:640] = p["v2"]
    b64[:, 640:655] = p["v3"]
    b64[0:1, 655:719] = p["d1l"]
    b64[0:1, 719:783] = p["d2l"]
    b64[0:1, 783:847] = p["d3l"]

    in_maps = []
    for c in range(N_CORES):
        xs = x[c * BL : (c + 1) * BL]
        # layout (c, h, w, b)
        x_adj = xs.transpose(1, 2, 3, 0).reshape(3, 64 * BL) - p["_c4mu"][:, None]
        b3f32 = np.zeros((3, 386), np.float32)
        b3f32[:, 0:128] = x_adj
        b3f32[:, 128:384] = p["v0"]
        b3f32[:, 384:385] = p["nc4lvh"]
        b3f32[:, 385:386] = p["c4lv"]
        in_maps.append(
            {
                "b3f16": b3f16,
                "b3f32": np.ascontiguousarray(b3f32),
                "b128": b128,
                "b64": b64,
            }
        )
    return in_maps


def kernel(**inputs):
    global _NC_CACHE, LAST_RESULT
    if _NC_CACHE is None:
        _NC_CACHE = build_nc()
    in_maps = make_in_maps(inputs)
    res = run_bass_kernel_spmd(
        _NC_CACHE, in_maps, core_ids=list(range(N_CORES)), trace=TRACE
    )
    LAST_RESULT = res
    ys, lss = [], []
    for c in range(N_CORES):
        ys.append(res.results[c]["out_y"].reshape(3, 8, 8, BL).transpose(3, 0, 1, 2))
        lss.append(res.results[c]["out_ls"].reshape(BL))
    return (
        np.ascontiguousarray(np.concatenate(ys), dtype=np.float32),
        np.ascontiguousarray(np.concatenate(lss), dtype=np.float32),
    )
